# revision 51
# baseline (speedup 1.0000x reference)
"""Binary conv2d (XNOR-style) + per-channel scale for Trainium2 — v10.

y = conv2d(sign(x), sign(w), stride=1, pad=1) * scale[oc]

Data-parallel over batch across 8 NeuronCores (4 images each).  The 3x3
conv over 256 in-channels is accumulating fp8 DoubleRow matmuls (K=256)
into a PSUM tile per 8-output-row chunk, using shifted windows of a
zero-padded 57-column-stride image.  PSUM accumulates in fp32 and all
matmul inputs are exactly representable, so the result is bit-identical
to the fp32 reference.

The matmul stream runs at ~97% of the fp8-DR peak (195 ns issue-to-issue
vs the 190 ns floor of 456 cycles @2.4 GHz, 157 TF/s), so this version
is entirely about starting that stream early and never letting it gap
(a PE idle gap also resets the 2.4 GHz clock to 1.2 GHz for ~3 us).

Trace-derived hardware facts this layout is built around:
  - Each HWDGE ring is a FIFO whose descriptors spread over all 16 DMA
    engines (~360 GB/s shared): a transfer completes when the ring has
    drained everything queued before it, so the two startup rings are
    ordered exactly by need (image-0 quarter-pieces first, weights
    interleaved behind piece 1).  The scale's DMA is 128 tiny
    descriptors (~0.9 us of FIFO) and rides the otherwise-idle gpsimd
    SWDGE ring.
  - The Tile scheduler hoists dependency-free dma_starts ahead of
    emission order, so queue position cannot pace a transfer.  Images
    1-3 are paced by GATES: a gpsimd copy that READS a region an
    earlier sign wrote and WRITES into the next load's DMA destination,
    giving that DMA a real dependency on the sign having completed
    (this also pins the ACT sign order, which the scheduler otherwise
    picks from its own wrong DMA-readiness estimates).
  - The PE clock ramps 1.2 -> 2.4 GHz only after ~3 us of continuous
    high-occupancy execution; K=256 DoubleRow warmup matmuls (low-K
    ones do NOT ramp it) with a 256-wide free dim (107 ns once ramped)
    run until piece-1's sign completes.
  - Image 0 is signed in four 14-row pieces (both ic slots per sign);
    compute group (0,3) is emitted right after piece 1's sign — at that
    point later signs don't exist, so dependency tracking cannot
    over-wait on them (the v1 kernel's first matmul waited on a third
    sign and started 3.3 us late).  Group (3,7) follows pieces 2-3 with
    ~7 us of slack.
"""

import numpy as np
import ml_dtypes

N_CORES = 8
IMGS = 4  # images per core
IC = 256
OC = 256
H = W = 56
# Padded row stride is 57, not 58: for a 3-wide kernel the left pad of
# row r+1 doubles as the right pad of row r, halving the dead columns.
WPAD = 57
XPAD_F = 3312  # 58 padded rows * 57 = 3306 -> pad to mult of 16
ROWS = 8  # output rows per PSUM tile
NFREE = ROWS * WPAD  # 456 <= 512 (PSUM bank limit)
NCHUNK = H // ROWS  # 7
QR = ((0, 14), (14, 28), (28, 56))  # image-0 pieces (14/14/28 rows)
N_WARM = 62  # PE warmups: ~3us ramp then 107ns each, end at data-ready
WFREE = 256  # warmup matmul free dim

_cache = {}


def _install_drain_patch():
    """This walrus build rejects >1 sync-wait on ctrl-type instructions;
    Tile's kernel-tail drain carries one wait per pending proc.  Split it
    into one drain per proc (each with <=1 wait)."""
    import concourse.tile as _tile
    from concourse.vector_clock import ScopedClock, VectorClock

    if getattr(_tile.TileContext, "_drain_split_patch", False):
        return

    def _drain_and_barrier(self, tick_clock, wait_clock):
        nc = self.nc
        gclock = tick_clock.global_clock
        n = len(gclock)
        for p in range(n):
            t = gclock[p]
            if t <= 0:
                continue
            vec = [0] * n
            vec[p] = t
            d = nc.gpsimd.drain()
            wait_clock.add_sem_waits(d.ins, ScopedClock({None: VectorClock(vec)}))
        assert self.sems is not None
        popped = nc._tile_sem_poison_stack.pop()
        assert popped is self._sem_poison
        nc.clear_and_free_semaphores(list(self.sems.allocated().values()))

    _tile.TileContext._drain_and_barrier = _drain_and_barrier
    _tile.TileContext._drain_split_patch = True


def _split_excess_waits(nc, maxw=1):
    """Same walrus limitation: hoist excess sync-waits onto same-engine
    NoOps inserted just before the instruction (engine streams are
    in-order, so a preceding NoOp carrying the waits is equivalent)."""
    import concourse.mybir as mybir

    n_split = 0
    for f in nc.m.functions:
        for bb in f.blocks:
            out = []
            for ins in bb.instructions:
                si = ins.sync_info
                if si and si.on_wait and len(si.on_wait) > maxw:
                    waits = list(si.on_wait)
                    excess, keep = waits[:-maxw], waits[-maxw:]
                    for i in range(0, len(excess), maxw):
                        nop = mybir.InstNoOp(
                            name=f"{ins.name}_waitsplit{i}",
                            engine=ins.engine,
                            ins=[],
                            outs=[],
                            sync_info=mybir.SyncInfo(
                                on_wait=excess[i : i + maxw], on_update=[]
                            ),
                        )
                        out.append(nop)
                    si.on_wait = keep
                    n_split += 1
                out.append(ins)
            bb.instructions = out
    return n_split


def build_nc():
    import concourse.bass as bass
    import concourse.mybir as mybir
    from concourse.tile import TileContext

    _install_drain_patch()

    f32 = mybir.dt.float32
    fp8 = mybir.dt.float8e4
    DR = mybir.MatmulPerfMode.DoubleRow

    nc = bass.Bass()
    x = nc.declare_dram_parameter("x", [IMGS, IC, H, W], f32, isOutput=False)
    wb8 = nc.declare_dram_parameter("wb8", [128, 18, OC], fp8, isOutput=False)
    sc2 = nc.declare_dram_parameter("sc2", [128, 2], f32, isOutput=False)
    y = nc.declare_dram_parameter("y", [IMGS, OC, H, W], f32, isOutput=True)

    with TileContext(nc) as tc:
        with (
            tc.tile_pool(name="const", bufs=1) as cpool,
            tc.tile_pool(name="xinq", bufs=len(QR)) as q_pool,
            tc.tile_pool(name="xin", bufs=2) as xin_pool,
            tc.tile_pool(name="outp", bufs=12) as out_pool,
            tc.tile_pool(name="psum", bufs=8, space="PSUM") as psum_pool,
        ):
            wb = cpool.tile([128, 18, OC], fp8)
            sc = cpool.tile([128, 2], f32)
            xp = cpool.tile([128, IMGS * 2, XPAD_F], fp8)
            wsc = cpool.tile([128, 2, 592], fp8)  # warmup scratch

            # warmup scratch memset split across the two idle engines so
            # the first warmup matmul issues early
            nc.gpsimd.memset(wsc[:, :, 0:296], 0.0)
            nc.vector.memset(wsc[:, :, 296:592], 0.0)

            q_tiles = [
                q_pool.tile(
                    [128, 2, QR[p][1] - QR[p][0], W], f32, name=f"xq{p}", tag="xinq"
                )
                for p in range(len(QR))
            ]

            # --- startup FIFOs, ordered by need.  slot 0 of each piece
            # on sync, slot 1 on scalar; weights interleave after piece 1
            # (taps 0-4 via sync, 5-8 via scalar); scale on gpsimd.
            def qload(p):
                r0, r1 = QR[p]
                nc.sync.dma_start(
                    out=q_tiles[p][:, 0, 0 : r1 - r0, :], in_=x[0, 0:128, r0:r1, :]
                )
                nc.scalar.dma_start(
                    out=q_tiles[p][:, 1, 0 : r1 - r0, :], in_=x[0, 128:256, r0:r1, :]
                )

            qload(0)
            qload(1)
            # weights split by PARTITION halves across both rings: ring
            # FIFO time is ~per-descriptor (one per partition), so 64-
            # descriptor halves land ~1us earlier than one 128-descriptor
            # transfer queued third on a single ring
            nc.sync.dma_start(out=wb[0:64, 0:10, :], in_=wb8[0:64, 0:10, :])
            nc.scalar.dma_start(out=wb[64:128, 0:10, :], in_=wb8[64:128, 0:10, :])
            nc.sync.dma_start(out=wb[0:64, 10:18, :], in_=wb8[0:64, 10:18, :])
            nc.scalar.dma_start(out=wb[64:128, 10:18, :], in_=wb8[64:128, 10:18, :])
            nc.scalar.dma_start(out=sc[:], in_=sc2[:, :])
            qload(2)

            # --- PE clock warmup (see module docstring)
            for k in range(N_WARM):
                ps = psum_pool.tile([128, NFREE], f32, name=f"warm{k}", tag="ps")
                nc.tensor.matmul(
                    ps[:, 0:WFREE], wsc[:, :, 456:584], wsc[:, :, 0:WFREE],
                    start=True, stop=True, perf_mode=DR,
                )

            def pad_ring(j):
                # zero only the padding ring (interior is overwritten by
                # the sign): top pad row; each data row's col 0 (also the
                # previous row's right pad); bottom pad row + tail.
                eng = nc.vector if j % 2 == 0 else nc.gpsimd
                xpj = xp[:, j, :]
                eng.memset(xpj[:, 0:WPAD], 0.0)
                lefts = xpj[:, WPAD : WPAD + H * WPAD].rearrange(
                    "p (r c) -> p r c", c=WPAD
                )[:, :, 0:1]
                eng.memset(lefts, 0.0)
                eng.memset(xpj[:, (H + 1) * WPAD : XPAD_F], 0.0)

            # ALL images' pad rings up front: dependency-free; emitting
            # them later would queue the vector half behind every earlier
            # drain on the DVE stream (measured 3.6us stall of image 1).
            for j in range(IMGS * 2):
                pad_ring(j)

            def sign_q(p):
                # binarize both ic-slots of one image-0 piece via the ACT
                # sign activation (signs own ACT; drains own DVE)
                r0, r1 = QR[p]
                base = (r0 + 1) * WPAD + 1
                dst = (
                    xp[:, 0:2, base : base + (r1 - r0) * WPAD]
                    .rearrange("p j (h w) -> p j h w", w=WPAD)[:, :, :, 0:W]
                )
                nc.scalar.sign(dst, q_tiles[p][:, :, 0 : r1 - r0, :])

            def sign_slot(j, xin):
                base = WPAD + 1
                dst = (
                    xp[:, j, base : base + H * WPAD]
                    .rearrange("p (h w) -> p h w", w=WPAD)[:, :, 0:W]
                )
                nc.scalar.sign(dst, xin[:])

            def sign_out(j, r0=0):
                # two elements of the xp region a sign wrote: reading them
                # creates a dependency on that sign having completed
                base = (r0 + 1) * WPAD + 1
                return xp[:, j, base : base + 2]

            def gate(next_tile, dep):
                # real WAW dependency pacing next_tile's DMAs behind `dep`
                nc.gpsimd.tensor_copy(next_tile[:, 0, 0:2], dep)

            def compute_image(n, subs):
                # tap-outer (weight-stationary) so consecutive matmuls hit
                # different PSUM banks.  LDWEIGHTS overlaps MATMUL via the
                # PE dual weight buffer.  flat 456-wide rhs windows: a
                # 3-dim rhs AP measures ~47ns/matmul slower, keep flat.
                for c0, c1 in subs:
                    for ocb in range(2):
                        psums = [
                            psum_pool.tile(
                                [128, NFREE], f32, name=f"ps{n}{ocb}{c}", tag="ps"
                            )
                            for c in range(c0, c1)
                        ]
                        for t in range(9):
                            kh, kw = divmod(t, 3)
                            lhsT = wb[:, 2 * t : 2 * t + 2, ocb * 128 : (ocb + 1) * 128]
                            rhs_slot = xp[:, 2 * n : 2 * n + 2, :]
                            for c in range(c0, c1):
                                off = c * ROWS * WPAD + kh * WPAD + kw
                                nc.tensor.matmul(
                                    psums[c - c0][:],
                                    lhsT,
                                    rhs_slot[:, :, off : off + NFREE],
                                    start=(t == 0),
                                    stop=(t == 8),
                                    perf_mode=DR,
                                )
                        for c in range(c0, c1):
                            out_c = out_pool.tile([128, ROWS, W], f32)
                            src = psums[c - c0].rearrange("p (h w) -> p h w", w=WPAD)[
                                :, :, 0:W
                            ]
                            # all drains on DVE (signs own ACT; Pool cannot
                            # read PSUM); the fp32 scale is applied here
                            nc.vector.tensor_scalar_mul(
                                out_c[:], src, sc[:, ocb : ocb + 1]
                            )
                            nc.sync.dma_start(
                                out=y[n, ocb * 128 : (ocb + 1) * 128, c * ROWS : (c + 1) * ROWS, :],
                                in_=out_c[:],
                            )

            # image 0: group (0,3) right after piece-1's sign (needs rows
            # <=24 (in pieces 0-1); emitting it before later signs exist
            # prevents dependency over-waits); (3,7) after pieces 2-3.
            sign_q(0)
            sign_q(1)
            compute_image(0, subs=((0, 3),))
            sign_q(2)

            def load_image(n, deps):
                # full-slot loads: slot 0 on sync, slot 1 on scalar, each
                # gated on an earlier sign's output so the transfers stay
                # out of the startup window and the ACT sign order is
                # pinned; the xin pool rotation adds back-pressure.
                for icb in range(2):
                    j = n * 2 + icb
                    xin = xin_pool.tile([128, H, W], f32, name=f"xin{j}", tag="xin")
                    gate(xin, deps[icb])
                    ring = nc.sync if icb == 0 else nc.scalar
                    ring.dma_start(
                        out=xin[:], in_=x[n, icb * 128 : (icb + 1) * 128, :, :]
                    )
                    sign_slot(j, xin)

            load_image(1, (sign_out(0, QR[2][0]), sign_out(1, QR[2][0])))
            compute_image(0, subs=((3, NCHUNK),))
            compute_image(1, subs=((0, 4),))
            load_image(2, (sign_out(2), sign_out(3)))
            compute_image(1, subs=((4, NCHUNK),))
            compute_image(2, subs=((0, 4),))
            load_image(3, (sign_out(4), sign_out(5)))
            compute_image(2, subs=((4, NCHUNK),))
            # final group is a single chunk so the post-stream drain+store
            # tail is as short as possible
            compute_image(3, subs=((0, 3), (3, 5), (5, 6), (6, NCHUNK)))

    _split_excess_waits(nc)
    return nc


def _get_nc():
    if "nc" not in _cache:
        _cache["nc"] = build_nc()
    return _cache["nc"]


def _prep_weights(weight, scale):
    # host-side: binarize weights, lay out [p, (kh kw icb), oc] fp8; the
    # per-channel scale is rearranged to [p, ocb].
    w = np.asarray(weight, dtype=np.float32)  # [oc, ic, kh, kw]
    wb = np.sign(w).transpose(2, 3, 1, 0)  # [kh, kw, ic, oc]
    wb = wb.reshape(3, 3, 2, 128, OC).transpose(3, 0, 1, 2, 4).reshape(128, 18, OC)
    wb8 = np.ascontiguousarray(wb).astype(ml_dtypes.float8_e4m3)
    sc2 = np.ascontiguousarray(np.asarray(scale, dtype=np.float32).reshape(2, 128).T)
    return wb8, sc2


def run(inputs, trace=False, trace_cores=None):
    from concourse.bass_utils import run_bass_kernel_spmd

    x = np.asarray(inputs["x"])
    wb8, sc2 = _prep_weights(inputs["weight"], inputs["scale"])

    in_maps = [
        {"x": x[i * IMGS : (i + 1) * IMGS], "wb8": wb8, "sc2": sc2}
        for i in range(N_CORES)
    ]
    res = run_bass_kernel_spmd(
        _get_nc(),
        in_maps,
        core_ids=list(range(N_CORES)),
        trace=trace,
        trace_cores=trace_cores,
    )
    out = np.concatenate([res.results[i]["y"] for i in range(N_CORES)], axis=0)
    return out, res


def kernel(**inputs):
    # One retry: a previously crashed process can leave a core wedged
    # (NRT_EXEC_UNIT_UNRECOVERABLE); the runtime recovers on the next
    # attempt.
    try:
        out, _ = run(inputs, trace=False)
    except Exception:
        out, _ = run(inputs, trace=False)
    return out


# revision 53
# speedup vs baseline: 1.0322x; 1.0322x over previous
"""Binary conv2d (XNOR-style) + per-channel scale for Trainium2 — v10.

y = conv2d(sign(x), sign(w), stride=1, pad=1) * scale[oc]

Data-parallel over batch across 8 NeuronCores (4 images each).  The 3x3
conv over 256 in-channels is accumulating fp8 DoubleRow matmuls (K=256)
into a PSUM tile per 8-output-row chunk, using shifted windows of a
zero-padded 57-column-stride image.  PSUM accumulates in fp32 and all
matmul inputs are exactly representable, so the result is bit-identical
to the fp32 reference.

The matmul stream runs at ~97% of the fp8-DR peak (195 ns issue-to-issue
vs the 190 ns floor of 456 cycles @2.4 GHz, 157 TF/s), so this version
is entirely about starting that stream early and never letting it gap
(a PE idle gap also resets the 2.4 GHz clock to 1.2 GHz for ~3 us).

Trace-derived hardware facts this layout is built around:
  - Each HWDGE ring is a FIFO whose descriptors spread over all 16 DMA
    engines (~360 GB/s shared): a transfer completes when the ring has
    drained everything queued before it, so the two startup rings are
    ordered exactly by need (image-0 quarter-pieces first, weights
    interleaved behind piece 1).  The scale's DMA is 128 tiny
    descriptors (~0.9 us of FIFO) and rides the otherwise-idle gpsimd
    SWDGE ring.
  - The Tile scheduler hoists dependency-free dma_starts ahead of
    emission order, so queue position cannot pace a transfer.  Images
    1-3 are paced by GATES: a gpsimd copy that READS a region an
    earlier sign wrote and WRITES into the next load's DMA destination,
    giving that DMA a real dependency on the sign having completed
    (this also pins the ACT sign order, which the scheduler otherwise
    picks from its own wrong DMA-readiness estimates).
  - The PE clock ramps 1.2 -> 2.4 GHz only after ~3 us of continuous
    high-occupancy execution; K=256 DoubleRow warmup matmuls (low-K
    ones do NOT ramp it) with a 256-wide free dim (107 ns once ramped)
    run until piece-1's sign completes.
  - Image 0 is signed in four 14-row pieces (both ic slots per sign);
    compute group (0,3) is emitted right after piece 1's sign — at that
    point later signs don't exist, so dependency tracking cannot
    over-wait on them (the v1 kernel's first matmul waited on a third
    sign and started 3.3 us late).  Group (3,7) follows pieces 2-3 with
    ~7 us of slack.
"""

import numpy as np
import ml_dtypes

N_CORES = 8
IMGS = 4  # images per core
IC = 256
OC = 256
H = W = 56
# Padded row stride is 57, not 58: for a 3-wide kernel the left pad of
# row r+1 doubles as the right pad of row r, halving the dead columns.
WPAD = 57
XPAD_F = 3312  # 58 padded rows * 57 = 3306 -> pad to mult of 16
ROWS = 8  # output rows per PSUM tile
NFREE = ROWS * WPAD  # 456 <= 512 (PSUM bank limit)
NCHUNK = H // ROWS  # 7
QR = ((0, 14), (14, 28), (28, 56))  # image-0 pieces (14/14/28 rows)
N_WARM = 62  # PE warmups: ~3us ramp then 107ns each, end at data-ready
WFREE = 256  # warmup matmul free dim

_cache = {}


def _install_drain_patch():
    """This walrus build rejects >1 sync-wait on ctrl-type instructions;
    Tile's kernel-tail drain carries one wait per pending proc.  Split it
    into one drain per proc (each with <=1 wait)."""
    import concourse.tile as _tile
    from concourse.vector_clock import ScopedClock, VectorClock

    if getattr(_tile.TileContext, "_drain_split_patch", False):
        return

    def _drain_and_barrier(self, tick_clock, wait_clock):
        nc = self.nc
        gclock = tick_clock.global_clock
        n = len(gclock)
        for p in range(n):
            t = gclock[p]
            if t <= 0:
                continue
            vec = [0] * n
            vec[p] = t
            d = nc.gpsimd.drain()
            wait_clock.add_sem_waits(d.ins, ScopedClock({None: VectorClock(vec)}))
        assert self.sems is not None
        popped = nc._tile_sem_poison_stack.pop()
        assert popped is self._sem_poison
        nc.clear_and_free_semaphores(list(self.sems.allocated().values()))

    _tile.TileContext._drain_and_barrier = _drain_and_barrier
    _tile.TileContext._drain_split_patch = True


def _split_excess_waits(nc, maxw=1):
    """Same walrus limitation: hoist excess sync-waits onto same-engine
    NoOps inserted just before the instruction (engine streams are
    in-order, so a preceding NoOp carrying the waits is equivalent)."""
    import concourse.mybir as mybir

    n_split = 0
    for f in nc.m.functions:
        for bb in f.blocks:
            out = []
            for ins in bb.instructions:
                si = ins.sync_info
                if si and si.on_wait and len(si.on_wait) > maxw:
                    waits = list(si.on_wait)
                    excess, keep = waits[:-maxw], waits[-maxw:]
                    for i in range(0, len(excess), maxw):
                        nop = mybir.InstNoOp(
                            name=f"{ins.name}_waitsplit{i}",
                            engine=ins.engine,
                            ins=[],
                            outs=[],
                            sync_info=mybir.SyncInfo(
                                on_wait=excess[i : i + maxw], on_update=[]
                            ),
                        )
                        out.append(nop)
                    si.on_wait = keep
                    n_split += 1
                out.append(ins)
            bb.instructions = out
    return n_split


def build_nc():
    import concourse.bass as bass
    import concourse.mybir as mybir
    from concourse.tile import TileContext

    _install_drain_patch()

    f32 = mybir.dt.float32
    fp8 = mybir.dt.float8e4
    DR = mybir.MatmulPerfMode.DoubleRow

    nc = bass.Bass()
    x = nc.declare_dram_parameter("x", [IMGS, IC, H, W], f32, isOutput=False)
    wb8 = nc.declare_dram_parameter("wb8", [128, 18, OC], fp8, isOutput=False)
    sc2 = nc.declare_dram_parameter("sc2", [128, 2], f32, isOutput=False)
    y = nc.declare_dram_parameter("y", [IMGS, OC, H, W], f32, isOutput=True)

    with TileContext(nc) as tc:
        with (
            tc.tile_pool(name="const", bufs=1) as cpool,
            tc.tile_pool(name="xinq", bufs=len(QR)) as q_pool,
            tc.tile_pool(name="xin", bufs=2) as xin_pool,
            tc.tile_pool(name="outp", bufs=12) as out_pool,
            tc.tile_pool(name="psum", bufs=8, space="PSUM") as psum_pool,
        ):
            wb = cpool.tile([128, 18, OC], fp8)
            sc = cpool.tile([128, 2], f32)
            xp = cpool.tile([128, IMGS * 2, XPAD_F], fp8)
            wsc = cpool.tile([128, 2, 592], fp8)  # warmup scratch

            # warmup scratch memset split across the two idle engines so
            # the first warmup matmul issues early
            nc.gpsimd.memset(wsc[:, :, 0:296], 0.0)
            nc.vector.memset(wsc[:, :, 296:592], 0.0)

            q_tiles = [
                q_pool.tile(
                    [128, 2, QR[p][1] - QR[p][0], W], f32, name=f"xq{p}", tag="xinq"
                )
                for p in range(len(QR))
            ]

            # --- startup FIFOs, ordered by need.  slot 0 of each piece
            # on sync, slot 1 on scalar; weights interleave after piece 1
            # (taps 0-4 via sync, 5-8 via scalar); scale on gpsimd.
            def qload(p):
                r0, r1 = QR[p]
                nc.sync.dma_start(
                    out=q_tiles[p][:, 0, 0 : r1 - r0, :], in_=x[0, 0:128, r0:r1, :]
                )
                nc.scalar.dma_start(
                    out=q_tiles[p][:, 1, 0 : r1 - r0, :], in_=x[0, 128:256, r0:r1, :]
                )

            qload(0)
            qload(1)
            nc.sync.dma_start(out=wb[:, 0:10, :], in_=wb8[:, 0:10, :])
            nc.scalar.dma_start(out=wb[:, 10:18, :], in_=wb8[:, 10:18, :])
            nc.gpsimd.dma_start(out=sc[:], in_=sc2[:, :])
            qload(2)

            # --- PE clock warmup (see module docstring)
            for k in range(N_WARM):
                ps = psum_pool.tile([128, NFREE], f32, name=f"warm{k}", tag="ps")
                nc.tensor.matmul(
                    ps[:, 0:WFREE], wsc[:, :, 456:584], wsc[:, :, 0:WFREE],
                    start=True, stop=True, perf_mode=DR,
                )

            def pad_ring(j):
                # zero only the padding ring (interior is overwritten by
                # the sign): top pad row; each data row's col 0 (also the
                # previous row's right pad); bottom pad row + tail.
                eng = nc.vector if j % 2 == 0 else nc.gpsimd
                xpj = xp[:, j, :]
                eng.memset(xpj[:, 0:WPAD], 0.0)
                lefts = xpj[:, WPAD : WPAD + H * WPAD].rearrange(
                    "p (r c) -> p r c", c=WPAD
                )[:, :, 0:1]
                eng.memset(lefts, 0.0)
                eng.memset(xpj[:, (H + 1) * WPAD : XPAD_F], 0.0)

            # ALL images' pad rings up front: dependency-free; emitting
            # them later would queue the vector half behind every earlier
            # drain on the DVE stream (measured 3.6us stall of image 1).
            for j in range(IMGS * 2):
                pad_ring(j)

            def sign_q(p):
                # binarize both ic-slots of one image-0 piece via the ACT
                # sign activation (signs own ACT; drains own DVE)
                r0, r1 = QR[p]
                base = (r0 + 1) * WPAD + 1
                dst = (
                    xp[:, 0:2, base : base + (r1 - r0) * WPAD]
                    .rearrange("p j (h w) -> p j h w", w=WPAD)[:, :, :, 0:W]
                )
                nc.scalar.sign(dst, q_tiles[p][:, :, 0 : r1 - r0, :])

            def sign_slot(j, xin):
                base = WPAD + 1
                dst = (
                    xp[:, j, base : base + H * WPAD]
                    .rearrange("p (h w) -> p h w", w=WPAD)[:, :, 0:W]
                )
                nc.scalar.sign(dst, xin[:])

            def sign_out(j, r0=0):
                # two elements of the xp region a sign wrote: reading them
                # creates a dependency on that sign having completed
                base = (r0 + 1) * WPAD + 1
                return xp[:, j, base : base + 2]

            def gate(next_tile, dep):
                # real WAW dependency pacing next_tile's DMAs behind `dep`
                nc.gpsimd.tensor_copy(next_tile[:, 0, 0:2], dep)

            def compute_image(n, subs):
                # tap-outer (weight-stationary) so consecutive matmuls hit
                # different PSUM banks.  LDWEIGHTS overlaps MATMUL via the
                # PE dual weight buffer.  flat 456-wide rhs windows: a
                # 3-dim rhs AP measures ~47ns/matmul slower, keep flat.
                for c0, c1 in subs:
                    for ocb in range(2):
                        psums = [
                            psum_pool.tile(
                                [128, NFREE], f32, name=f"ps{n}{ocb}{c}", tag="ps"
                            )
                            for c in range(c0, c1)
                        ]
                        for t in range(9):
                            kh, kw = divmod(t, 3)
                            lhsT = wb[:, 2 * t : 2 * t + 2, ocb * 128 : (ocb + 1) * 128]
                            rhs_slot = xp[:, 2 * n : 2 * n + 2, :]
                            for c in range(c0, c1):
                                off = c * ROWS * WPAD + kh * WPAD + kw
                                nc.tensor.matmul(
                                    psums[c - c0][:],
                                    lhsT,
                                    rhs_slot[:, :, off : off + NFREE],
                                    start=(t == 0),
                                    stop=(t == 8),
                                    perf_mode=DR,
                                )
                        for c in range(c0, c1):
                            out_c = out_pool.tile([128, ROWS, W], f32)
                            src = psums[c - c0].rearrange("p (h w) -> p h w", w=WPAD)[
                                :, :, 0:W
                            ]
                            # all drains on DVE (signs own ACT; Pool cannot
                            # read PSUM); the fp32 scale is applied here
                            nc.vector.tensor_scalar_mul(
                                out_c[:], src, sc[:, ocb : ocb + 1]
                            )
                            nc.sync.dma_start(
                                out=y[n, ocb * 128 : (ocb + 1) * 128, c * ROWS : (c + 1) * ROWS, :],
                                in_=out_c[:],
                            )

            # image 0: group (0,3) right after piece-1's sign (needs rows
            # <=24 (in pieces 0-1); emitting it before later signs exist
            # prevents dependency over-waits); (3,7) after pieces 2-3.
            sign_q(0)
            sign_q(1)
            compute_image(0, subs=((0, 3),))
            sign_q(2)
            # keep-alives bridging the band-1 sign: one DEDICATED psum
            # tile reused by every filler matmul (no pool rotation, no
            # reader) so the PE stays hot instead of idling 3.5us and
            # resetting the 2.4GHz clock while sign-q2 completes
            kps = psum_pool.tile([128, NFREE], f32, name="keepps", tag="ps")
            for _ in range(26):
                nc.tensor.matmul(
                    kps[:, 0:WFREE], wsc[:, :, 456:584], wsc[:, :, 0:WFREE],
                    start=True, stop=True, perf_mode=DR,
                )

            def load_image(n, deps):
                # full-slot loads: slot 0 on sync, slot 1 on scalar, each
                # gated on an earlier sign's output so the transfers stay
                # out of the startup window and the ACT sign order is
                # pinned; the xin pool rotation adds back-pressure.
                for icb in range(2):
                    j = n * 2 + icb
                    xin = xin_pool.tile([128, H, W], f32, name=f"xin{j}", tag="xin")
                    gate(xin, deps[icb])
                    ring = nc.sync if icb == 0 else nc.scalar
                    ring.dma_start(
                        out=xin[:], in_=x[n, icb * 128 : (icb + 1) * 128, :, :]
                    )
                    sign_slot(j, xin)

            load_image(1, (sign_out(0, QR[2][0]), sign_out(1, QR[2][0])))
            compute_image(0, subs=((3, NCHUNK),))
            compute_image(1, subs=((0, 4),))
            load_image(2, (sign_out(2), sign_out(3)))
            compute_image(1, subs=((4, NCHUNK),))
            compute_image(2, subs=((0, 4),))
            load_image(3, (sign_out(4), sign_out(5)))
            compute_image(2, subs=((4, NCHUNK),))
            # final group is a single chunk so the post-stream drain+store
            # tail is as short as possible
            compute_image(3, subs=((0, 3), (3, 5), (5, 6), (6, NCHUNK)))

    _split_excess_waits(nc)
    return nc


def _get_nc():
    if "nc" not in _cache:
        _cache["nc"] = build_nc()
    return _cache["nc"]


def _prep_weights(weight, scale):
    # host-side: binarize weights, lay out [p, (kh kw icb), oc] fp8; the
    # per-channel scale is rearranged to [p, ocb].
    w = np.asarray(weight, dtype=np.float32)  # [oc, ic, kh, kw]
    wb = np.sign(w).transpose(2, 3, 1, 0)  # [kh, kw, ic, oc]
    wb = wb.reshape(3, 3, 2, 128, OC).transpose(3, 0, 1, 2, 4).reshape(128, 18, OC)
    wb8 = np.ascontiguousarray(wb).astype(ml_dtypes.float8_e4m3)
    sc2 = np.ascontiguousarray(np.asarray(scale, dtype=np.float32).reshape(2, 128).T)
    return wb8, sc2


def run(inputs, trace=False, trace_cores=None):
    from concourse.bass_utils import run_bass_kernel_spmd

    x = np.asarray(inputs["x"])
    wb8, sc2 = _prep_weights(inputs["weight"], inputs["scale"])

    in_maps = [
        {"x": x[i * IMGS : (i + 1) * IMGS], "wb8": wb8, "sc2": sc2}
        for i in range(N_CORES)
    ]
    res = run_bass_kernel_spmd(
        _get_nc(),
        in_maps,
        core_ids=list(range(N_CORES)),
        trace=trace,
        trace_cores=trace_cores,
    )
    out = np.concatenate([res.results[i]["y"] for i in range(N_CORES)], axis=0)
    return out, res


def kernel(**inputs):
    # One retry: a previously crashed process can leave a core wedged
    # (NRT_EXEC_UNIT_UNRECOVERABLE); the runtime recovers on the next
    # attempt.
    try:
        out, _ = run(inputs, trace=False)
    except Exception:
        out, _ = run(inputs, trace=False)
    return out
